# revision 14
# baseline (speedup 1.0000x reference)
"""Trainium2 Bass kernel for nn_Attention_sep — v2 (bf16, SBUF-resident K/V).

Sharding: query rows split across 8 cores (528 patch + 16 det queries per
core, zero-padded); K/V projections replicated per core. All matmul operands
are bf16 (fp32 PSUM accumulation); K^T (feature-major) and V (token-major,
ones column appended per head for sumexp) live entirely in SBUF as
per-superblock tiles — no DRAM scratch roundtrip. The K/V projection of
superblock j+1 is interleaved into the attention pj-loop of superblock j
(sharing PSUM banks in pool rotation) so the PE never drains. Attention runs
keys-major: S^T = K_h^T'Q_h^T per 128-key chunk, exp(SCALE*s) on ScalarE
from PSUM to bf16, attn@V accumulates o^T + sumexp in PSUM over each
superblock, flushed to an SBUF accumulator on DVE (GPSIMD cannot touch
PSUM). attn@V is software-pipelined two quanta behind the S matmuls so the
in-order PE never stalls on an exp that is not ready. The tail transposes
heads token-major, divides by sumexp, applies LayerNorm (bn_stats/bn_aggr,
exact eps; ln_g/ln_b are folded into the output weights/bias host-side),
transposes back, and projects (patch/det weights per segment). Host only
transposes/casts inputs and gathers per-core outputs.
"""
import sys
sys.path.insert(0, "/opt/trn_rl_repo")
import numpy as np

N_TOK = 4301
D = 768
H = 12
HD = 64
NDET = 100
NPATCH = N_TOK - NDET          # 4201
SCALE = HD ** -0.5
EPS = 1e-5
NCORES = 8
PQ = 528                        # per-core patch queries (528*8 = 4224 >= 4201)
DQ = 16                         # per-core det queries (16*8 = 128 >= 100)
TQ = PQ + DQ                    # 544
QB = TQ // 2                    # 272 (one PSUM bank per q-block)
DC = D // 128                   # 6 feature/contraction chunks

# key chunks: 32 x 128 patch, 105 patch tail, 100 det  (exactly 4301 keys)
KC_SIZES = [128] * 32 + [105, 100]
KC_STARTS = [128 * i for i in range(32)] + [4096, 4201]
NKC = len(KC_SIZES)             # 34
# superblocks: small first sb so attention starts early, then 768/1024s
SB_CHUNKS = ([[0, 1], [2, 3, 4, 5, 6, 7]] +
             [list(range(8 * j, 8 * j + 8)) for j in range(1, 4)] +
             [[32], [33]])
SB_START = [KC_STARTS[c[0]] for c in SB_CHUNKS]
SB_SIZE = [sum(KC_SIZES[i] for i in c) for c in SB_CHUNKS]
NSB = len(SB_CHUNKS)            # 10

_CACHE = {}


def _build():
    import concourse.bass as bass
    import concourse.tile as tile
    from concourse import bacc, mybir
    from concourse.masks import make_identity

    FP32 = mybir.dt.float32
    BF16 = mybir.dt.bfloat16
    AF = mybir.ActivationFunctionType
    ALU = mybir.AluOpType

    nc = bacc.Bacc(name="attn_sep2")

    def din(name, shape, dt=FP32):
        return nc.dram_tensor(name, shape, dt, kind="ExternalInput")

    xT = din("xT", [D, N_TOK], BF16)
    xqT = din("xqT", [D, TQ], BF16)
    w_in = {k: din(k, [D, D], BF16) for k in
            ["wqT_p", "wqT_d", "wkT_p", "wkT_d", "wvT_p", "wvT_d",
             "woT_p", "woT_d"]}
    b_in = {k: din(k, [D]) for k in
            ["bq_p", "bq_d", "bv_p", "bv_d", "bo_p", "bo_d"]}
    outT = nc.dram_tensor("outT", [D, TQ], FP32, kind="ExternalOutput")
    outT_v = outT.rearrange("(c p) q -> p c q", p=128)
    xT_v = xT.rearrange("(c p) n -> p c n", p=128)
    xqT_v = xqT.rearrange("(c p) n -> p c n", p=128)

    from contextlib import ExitStack
    with tile.TileContext(nc) as tc:
        with ExitStack() as ctx:
            ep = ctx.enter_context
            qtp = ep(tc.tile_pool(name="qtp", bufs=1))
            wp = ep(tc.tile_pool(name="wp", bufs=2))
            xp = ep(tc.tile_pool(name="xp", bufs=2))
            kvp = ep(tc.tile_pool(name="kvp", bufs=1))
            ptp = ep(tc.tile_pool(name="ptp", bufs=4))
            oap = ep(tc.tile_pool(name="oap", bufs=1))
            asp = ep(tc.tile_pool(name="asp", bufs=1))
            onp = ep(tc.tile_pool(name="onp", bufs=2))
            oup = ep(tc.tile_pool(name="oup", bufs=2))
            sgl = ep(tc.tile_pool(name="sgl", bufs=1))
            sml = ep(tc.tile_pool(name="sml", bufs=1))
            psb = ep(tc.tile_pool(name="psb", bufs=4, space="PSUM"))
            drp = ep(tc.tile_pool(name="drp", bufs=1, space="DRAM"))
            pss = ep(tc.tile_pool(name="pss", bufs=2, space="PSUM"))
            # ---- constants / broadcast tiles ----
            ident = sgl.tile([128, 128], FP32, tag="ident")
            make_identity(nc, ident)

            def bcast(name):
                t = sgl.tile([128, D], BF16, tag=f"bc_{name}")
                src = b_in[name][:]
                nc.gpsimd.dma_start(
                    out=t,
                    in_=bass.AP(tensor=src.tensor, offset=src.offset,
                                ap=[[0, 128]] + [list(a) for a in src.ap]))
                return t

            bv_p_b = bcast("bv_p")
            bv_d_b = bcast("bv_d")

            def perpart(name):
                t = sgl.tile([128, DC], FP32, tag=f"pp_{name}")
                nc.sync.dma_start(t, b_in[name].rearrange("(c p) -> p c", p=128))
                return t

            eps_t = sgl.tile([128, 1], FP32, tag="eps")
            nc.vector.memset(eps_t, EPS)
            bq_p_s = perpart("bq_p")
            bq_d_s = perpart("bq_d")
            bo_p_s = perpart("bo_p")
            bo_d_s = perpart("bo_d")

            def load_w(name):
                t = wp.tile([128, DC, D], BF16, tag="w")
                nc.sync.dma_start(t, w_in[name].rearrange("(c p) f -> p c f", p=128))
                return t

            # ---- resident tensors ----
            QT = qtp.tile([128, DC, TQ], BF16, tag="QT")
            Ksb = [kvp.tile([128, DC, SB_SIZE[j] + SB_SIZE[j] % 2], BF16,
                            tag=f"k{j}", name=f"ksb{j}")
                   for j in range(NSB)]
            Vsb = [kvp.tile([128, len(SB_CHUNKS[j]), H, HD + 1], BF16,
                            tag=f"v{j}", name=f"vsb{j}")
                   for j in range(NSB)]

            # =========== Q^T projection (feature-major, bias added) ===========
            xq = xp.tile([128, DC, TQ], BF16, tag="x", name="xq0")
            nc.sync.dma_start(xq, xqT_v)
            wq_p = load_w("wqT_p")
            wq_d = load_w("wqT_d")
            q_segs = [(0, QB, wq_p, bq_p_s), (QB, PQ - QB, wq_p, bq_p_s),
                      (PQ, DQ, wq_d, bq_d_s)]
            for fc in range(DC):
                for c0, n, wq, bq in q_segs:
                    pq = psb.tile([128, 512], FP32, tag="bank")
                    for dc in range(DC):
                        nc.tensor.matmul(
                            pq[:, :n],
                            wq[:, dc, 128 * fc:128 * (fc + 1)],
                            xq[:, dc, c0:c0 + n],
                            start=(dc == 0), stop=(dc == DC - 1))
                    nc.vector.tensor_scalar_add(
                        QT[:, fc, c0:c0 + n], pq[:, :n], bq[:, fc:fc + 1])

            # ones columns of V (written once; V proj only touches cols 0:64)
            for j in range(NSB):
                nc.vector.memset(Vsb[j][:, :, :, HD:HD + 1], 1.0)

            # =========== K/V projection jobs (interleaved into attention) ====
            wk = {"p": load_w("wkT_p"), "d": None}
            wv = {"p": load_w("wvT_p"), "d": None}

            def proj_jobs(j):
                """Jobs that project superblock j's K^T and V into SBUF."""
                n0, sz = SB_START[j], SB_SIZE[j]
                sze = sz + sz % 2
                pd = "d" if j == NSB - 1 else "p"
                jobs = []

                def load_weights():
                    wk[pd] = load_w(f"wkT_{pd}")
                    wv[pd] = load_w(f"wvT_{pd}")
                if wk[pd] is None and pd == "p":
                    load_weights()          # first patch sb: load inline
                elif wk[pd] is None:
                    jobs.append(load_weights)

                bvb = bv_d_b if pd == "d" else bv_p_b
                xts = {}
                for hf0 in range(0, sz, 512):
                    hsz = min(512, sz - hf0)
                    hsze = hsz + hsz % 2

                    def load_x(hf0=hf0, hsze=hsze):
                        xt = xp.tile([128, DC, 512], BF16, tag="x",
                                     name=f"x{j}_{hf0}")
                        nc.sync.dma_start(
                            xt[:, :, :hsze], xT_v[:, :, n0 + hf0:n0 + hf0 + hsze])
                        xts[hf0] = xt
                    jobs.append(load_x)

                    def k_job(fc, hf0=hf0, hsz=hsz, hsze=hsze):
                        def run():
                            pk = psb.tile([128, 512], FP32, tag="bank")
                            for dc in range(DC):
                                nc.tensor.matmul(
                                    pk[:, :hsze],
                                    wk[pd][:, dc, 128 * fc:128 * (fc + 1)],
                                    xts[hf0][:, dc, :hsze],
                                    start=(dc == 0), stop=(dc == DC - 1))
                            nc.vector.tensor_copy(
                                Ksb[j][:, fc, hf0:hf0 + hsz], pk[:, :hsz])
                        return run
                    jobs += [k_job(fc) for fc in range(DC)]

                    def v_job(ci, half, hf0=hf0, hsz=hsz):
                        s0 = 128 * ci
                        m = min(128, hsz - s0)
                        f0 = half * 384
                        cg = (hf0 + s0) // 128

                        def run():
                            pv = psb.tile([128, 512], FP32, tag="bank")
                            for dc in range(DC):
                                nc.tensor.matmul(
                                    pv[:m, :384],
                                    xts[hf0][:, dc, s0:s0 + m],
                                    wv[pd][:, dc, f0:f0 + 384],
                                    start=(dc == 0), stop=(dc == DC - 1))
                            nc.vector.tensor_tensor(
                                Vsb[j][:m, cg, 6 * half:6 * (half + 1), :HD],
                                pv[:m, :384].rearrange("p (h d) -> p h d", d=HD),
                                bvb[:m, f0:f0 + 384].rearrange(
                                    "p (h d) -> p h d", d=HD),
                                ALU.add)
                        return run
                    jobs += [v_job(ci, half)
                             for ci in range((hsz + 127) // 128)
                             for half in range(2)]
                return jobs

            # =========== attention: sb-outer, pj-inner, proj interleaved =====
            oaccs = [oap.tile([65, H, QB], FP32, tag=f"oacc{qb}",
                              name=f"oacc{qb}")
                     for qb in range(2)]
            pending = proj_jobs(0)
            while pending:
                pending.pop(0)()
            flush_n = [0]
            for sbj in range(NSB):
                chunks = SB_CHUNKS[sbj]
                nch = len(chunks)
                k0_sb = KC_STARTS[chunks[0]]
                pending = proj_jobs(sbj + 1) if sbj + 1 < NSB else []
                per_pj = (len(pending) + DC - 1) // DC if pending else 0
                for pj in range(DC):
                    po = [psb.tile([65, QB], FP32, tag="bank",
                                   name=f"po{qb}{par}")
                          for qb in range(2) for par in range(2)]
                    # software-pipelined: attnV of quantum n is emitted after
                    # the S matmuls of quantum n+1 so the in-order PE never
                    # stalls at an attnV whose exp isn't ready yet
                    pending_av = []

                    def emit_av():
                        ci, qb, pt = pending_av.pop(0)
                        kc = KC_SIZES[chunks[ci]]
                        for par in range(2):
                            h = 2 * pj + par
                            nc.tensor.matmul(
                                po[2 * qb + par],
                                Vsb[sbj][:kc, ci, h, :],
                                pt[:kc, par, :],
                                start=(ci == 0), stop=(ci == nch - 1))
                    for ci, ch in enumerate(chunks):
                        kc = KC_SIZES[ch]
                        lk0 = KC_STARTS[ch] - k0_sb
                        for qb in range(2):
                            q0 = qb * QB
                            ps = pss.tile([128, 2, 512], FP32, tag="s2")
                            for par in range(2):
                                pb = 64 * par
                                nc.tensor.matmul(
                                    ps[:kc, par, :QB],
                                    Ksb[sbj][pb:pb + 64, pj, lk0:lk0 + kc],
                                    QT[pb:pb + 64, pj, q0:q0 + QB],
                                    start=True, stop=True)
                            pt = ptp.tile([128, 2, QB], BF16, tag="pt")
                            nc.scalar.activation(
                                pt[:kc], ps[:kc, :, :QB], AF.Exp, scale=SCALE)
                            pending_av.append((ci, qb, pt))
                            while len(pending_av) > 3:
                                emit_av()
                    while pending_av:
                        emit_av()
                    for qb in range(2):
                        for par in range(2):
                            h = 2 * pj + par
                            flush_n[0] += 1
                            if sbj == 0:
                                nc.vector.tensor_copy(
                                    oaccs[qb][:, h, :], po[2 * qb + par])
                            else:
                                nc.vector.tensor_tensor(
                                    oaccs[qb][:, h, :], oaccs[qb][:, h, :],
                                    po[2 * qb + par], ALU.add)
                    for _ in range(per_pj):
                        if pending:
                            pending.pop(0)()
                while pending:
                    pending.pop(0)()

            # ===== LN + out-proj (matmul stats; ln_g/ln_b folded into wo/bo)
            wo_p = load_w("woT_p")
            wo_d = load_w("woT_d")
            ones_t = sgl.tile([128, 1], BF16, tag="ones")
            nc.vector.memset(ones_t, 1.0)

            def pbcast(dst, src_t, shape, tag):
                dr = drp.tile(shape, BF16, tag=tag, name=f"dr_{tag}")
                nc.gpsimd.dma_start(dr, src_t)
                src = dr[:]
                nc.gpsimd.dma_start(
                    out=dst,
                    in_=bass.AP(tensor=src.tensor, offset=src.offset,
                                ap=[[0, dst.shape[0]]] + [list(a) for a in src.ap]))

            def tail_steps(qb):
                """Tail for one q-block: transpose heads token-major, divide
                by sumexp, LayerNorm stats (bn_stats/bn_aggr, exact eps),
                normalize, transpose back, project."""
                q0 = qb * QB
                oacc = oaccs[qb]
                o_lnT = onp.tile([128, DC, QB], BF16, tag="on",
                                 name=f"olnT{qb}")
                for gi, (off, L) in enumerate(
                        [(0, 128), (128, 128), (256, QB - 256)]):
                    # SBUF-only elementwise work alternates onto the (other-
                    # wise idle) Pool engine; PSUM reads must stay on DVE
                    pe2 = nc.gpsimd if (2 * qb + gi) % 2 == 0 else nc.vector
                    # normalize straight out of PSUM: per-head reciprocal of
                    # the sumexp column, then a scaled copy (ScalarE shares
                    # the work with DVE; both may read PSUM)
                    rs = sml.tile([128, H], FP32, tag="rs", name=f"rs{qb}{gi}")
                    o_n = onp.tile([128, D], FP32, tag="onn",
                                   name=f"on{qb}{gi}")
                    for h in range(H):
                        tp = psb.tile([128, 512], FP32, tag="bank")
                        nc.tensor.transpose(
                            tp[:L, :65], oacc[:, h, off:off + L],
                            ident[:65, :65])
                        nc.vector.reciprocal(rs[:L, h:h + 1],
                                             tp[:L, HD:HD + 1])
                        if h % 2 == 0:
                            nc.scalar.mul(
                                o_n[:L, HD * h:HD * (h + 1)],
                                tp[:L, :HD], rs[:L, h:h + 1])
                        else:
                            nc.vector.tensor_scalar_mul(
                                o_n[:L, HD * h:HD * (h + 1)],
                                tp[:L, :HD], rs[:L, h:h + 1])
                    yield
                    stats = sml.tile([128, 3, 6], FP32, tag="st",
                                     name=f"st{qb}{gi}")
                    for gj in range(3):
                        nc.vector.bn_stats(
                            stats[:L, gj], o_n[:L, 256 * gj:256 * (gj + 1)])
                    mv = sml.tile([128, 2], FP32, tag="mv", name=f"mv{qb}{gi}")
                    nc.vector.bn_aggr(mv[:L], stats[:L])
                    rstd = sml.tile([128, 1], FP32, tag="rstd",
                                    name=f"rstd{qb}{gi}")
                    nc.scalar.activation(rstd[:L], mv[:L, 1:2], AF.Sqrt,
                                         bias=eps_t[:L])
                    nc.vector.reciprocal(rstd[:L], rstd[:L])
                    nc.vector.tensor_scalar(
                        o_n[:L], o_n[:L], mv[:L, 0:1], rstd[:L],
                        ALU.subtract, ALU.mult)
                    yield
                    for fc in range(DC):
                        tp = psb.tile([128, 512], FP32, tag="bank")
                        nc.tensor.transpose(
                            tp[:, :L], o_n[:L, 128 * fc:128 * (fc + 1)],
                            ident[:L, :L])
                        if fc % 2 == 0:
                            nc.vector.tensor_copy(
                                o_lnT[:, fc, off:off + L], tp[:, :L])
                        else:
                            nc.scalar.copy(
                                o_lnT[:, fc, off:off + L], tp[:, :L])
                    yield

                # ---- output projection for this q-block ----
                if qb == 0:
                    segs = [(0, QB, wo_p, bo_p_s)]
                else:
                    segs = [(QB, PQ - QB, wo_p, bo_p_s), (PQ, DQ, wo_d, bo_d_s)]
                for fc in range(DC):
                    for c0, n, wo, bo in segs:
                        pu = psb.tile([128, 512], FP32, tag="bank")
                        for dc in range(DC):
                            nc.tensor.matmul(
                                pu[:, :n],
                                wo[:, dc, 128 * fc:128 * (fc + 1)],
                                o_lnT[:, dc, c0 - q0:c0 - q0 + n],
                                start=(dc == 0), stop=(dc == DC - 1))
                        ou = oup.tile([128, QB], FP32, tag="ou")
                        nc.vector.tensor_scalar_add(
                            ou[:, :n], pu[:, :n], bo[:, fc:fc + 1])
                        nc.sync.dma_start(outT_v[:, fc, c0:c0 + n], ou[:, :n])
                    yield

            gens = [tail_steps(0), tail_steps(1)]
            alive = [True, True]
            while any(alive):
                for gi2 in range(2):
                    if alive[gi2]:
                        try:
                            next(gens[gi2])
                        except StopIteration:
                            alive[gi2] = False

    nc.compile()
    return nc


def _run_spmd_dedup(nc, shared, percore):
    """Dispatch the prebuilt Bass module on 8 cores via PJRT.

    Shared inputs are uploaded sharded (1x wire traffic) and replicated
    on-device; donated output buffers are created on-device. Device-resident
    replicas are cached by content hash across calls."""
    import zlib
    import jax
    import jax.numpy as jnp
    from jax.experimental.shard_map import shard_map
    from jax.sharding import Mesh, PartitionSpec as P, NamedSharding
    from concourse import bass2jax, mybir

    bass2jax.install_neuronx_cc_hook()
    partition_name = (nc.partition_id_tensor.name
                      if nc.partition_id_tensor else None)
    in_names, out_names, out_avals = [], [], []
    for alloc in nc.m.functions[0].allocations:
        if not isinstance(alloc, mybir.MemoryLocationSet):
            continue
        name = alloc.memorylocations[0].name
        if alloc.kind == "ExternalInput":
            if name != partition_name:
                in_names.append(name)
        elif alloc.kind == "ExternalOutput":
            out_names.append(name)
            shape = tuple(alloc.tensor_shape)
            out_avals.append(jax.core.ShapedArray(shape, mybir.dt.np(alloc.dtype)))
    n_params = len(in_names)
    all_names = in_names + out_names
    if partition_name is not None:
        all_names = all_names + [partition_name]

    def _body(*args):
        ops = list(args)
        if partition_name is not None:
            ops.append(bass2jax.partition_id_tensor())
        outs = bass2jax._bass_exec_p.bind(
            *ops, out_avals=tuple(out_avals), in_names=tuple(all_names),
            out_names=tuple(out_names), lowering_input_output_aliases=(),
            sim_require_finite=True, sim_require_nnan=True, nc=nc)
        return tuple(outs)

    devices = jax.devices()[:NCORES]
    mesh = Mesh(np.asarray(devices), ("core",))
    rep = NamedSharding(mesh, P(None))
    shd = NamedSharding(mesh, P("core"))
    in_specs = tuple(P(None) if n in shared else P("core") for n in in_names) \
        + (P("core"),) * len(out_names)
    out_specs = (P("core"),) * len(out_names)
    donate = tuple(range(n_params, n_params + len(out_names)))
    if "jit_fn" not in _CACHE:
        _CACHE["jit_fn"] = jax.jit(
            shard_map(_body, mesh=mesh, in_specs=in_specs,
                      out_specs=out_specs, check_rep=False),
            donate_argnums=donate, keep_unused=True)
        _CACHE["replicate"] = jax.jit(lambda a: a, out_shardings=rep)
        _CACHE["dev_cache"] = {}

    def dev_shared(name, arr):
        key = (name, arr.shape, zlib.adler32(arr.tobytes()))
        c = _CACHE["dev_cache"]
        if c.get(name, (None, None))[0] == key:
            return c[name][1]
        a_sh = jax.device_put(arr, shd)        # 1x wire traffic
        a_rep = _CACHE["replicate"](a_sh)      # on-device all-gather
        c[name] = (key, a_rep)
        return a_rep

    zeros_fn = _CACHE.setdefault("zeros_fn", jax.jit(
        lambda: tuple(jnp.zeros((NCORES * a.shape[0], *a.shape[1:]), a.dtype)
                      for a in out_avals),
        out_shardings=tuple(shd for _ in out_avals)))

    ins = [dev_shared(n, shared[n]) if n in shared else
           jax.device_put(np.concatenate(percore[n], axis=0), shd)
           for n in in_names]
    zouts = zeros_fn()
    out_arrs = _CACHE["jit_fn"](*ins, *zouts)
    return [
        {name: np.asarray(out_arrs[i]).reshape(NCORES, *out_avals[i].shape)[c]
         for i, name in enumerate(out_names)}
        for c in range(NCORES)
    ]


def _run_multicore_sim(nc, in_maps):
    """Direct in-process 8-core interpreter execution (same engine the
    axon fake-nrt path uses), bypassing the PJRT plumbing."""
    from concourse.bass2jax import MultiCoreSim

    if not _CACHE.get("barrier_inserted"):
        nc.insert_bir_kernel_barrier_sem_inc()
        _CACHE["barrier_inserted"] = True
    sim = MultiCoreSim(nc, NCORES, aliases={},
                       require_finite=True, require_nnan=True)
    for c in range(NCORES):
        for name, arr in in_maps[c].items():
            sim.cores[c].tensor(name)[:] = arr
    sim.simulate()
    return [{"outT": np.array(sim.cores[c].tensor("outT"))}
            for c in range(NCORES)]


def kernel(**inputs):
    import ml_dtypes
    from concourse import bass_utils

    if "nc" not in _CACHE:
        _CACHE["nc"] = _build()
    nc = _CACHE["nc"]

    BF = ml_dtypes.bfloat16
    f = {k: np.ascontiguousarray(np.asarray(v, dtype=np.float32))
         for k, v in inputs.items()}
    x = f["x"][0]                                   # [4301, 768]
    xT = np.ascontiguousarray(x.T)                  # [768, 4301]

    base = {
        "xT": xT.astype(BF),
        "wqT_p": np.ascontiguousarray(f["wq_p"].T).astype(BF),
        "wqT_d": np.ascontiguousarray(f["wq_d"].T).astype(BF),
        "wkT_p": np.ascontiguousarray(f["wk_p"].T).astype(BF),
        "wkT_d": np.ascontiguousarray(f["wk_d"].T).astype(BF),
        "wvT_p": np.ascontiguousarray(f["wv_p"].T).astype(BF),
        "wvT_d": np.ascontiguousarray(f["wv_d"].T).astype(BF),
        "woT_p": np.ascontiguousarray((f["wo_p"] * f["ln_g"]).T).astype(BF),
        "woT_d": np.ascontiguousarray((f["wo_d"] * f["ln_g"]).T).astype(BF),
        "bq_p": f["bq_p"], "bq_d": f["bq_d"],
        "bv_p": f["bv_p"], "bv_d": f["bv_d"],
        "bo_p": f["bo_p"] + f["wo_p"] @ f["ln_b"],
        "bo_d": f["bo_d"] + f["wo_d"] @ f["ln_b"],
    }
    in_maps = []
    for c in range(NCORES):
        xqT = np.zeros((D, TQ), np.float32)
        p0, p1 = PQ * c, min(PQ * (c + 1), NPATCH)
        if p1 > p0:
            xqT[:, :p1 - p0] = xT[:, p0:p1]
        d0, d1 = DQ * c, min(DQ * (c + 1), NDET)
        if d1 > d0:
            xqT[:, PQ:PQ + d1 - d0] = xT[:, NPATCH + d0:NPATCH + d1]
        in_maps.append({**base, "xqT": np.ascontiguousarray(xqT.astype(BF))})

    try:
        if _CACHE.get("pjrt_broken"):
            raise RuntimeError("pjrt path disabled")
        results = _run_spmd_dedup(
            nc, shared=base,
            percore={"xqT": [m["xqT"] for m in in_maps]})
    except Exception:
        _CACHE.pop("jit_fn", None)
        _CACHE["pjrt_broken"] = True
        results = _run_multicore_sim(nc, in_maps)

    out = np.empty((N_TOK, D), np.float32)
    for c in range(NCORES):
        oc = results[c]["outT"].T                   # [544, 768]
        p0, p1 = PQ * c, min(PQ * (c + 1), NPATCH)
        if p1 > p0:
            out[p0:p1] = oc[:p1 - p0]
        d0, d1 = DQ * c, min(DQ * (c + 1), NDET)
        if d1 > d0:
            out[NPATCH + d0:NPATCH + d1] = oc[PQ:PQ + d1 - d0]
    return out[None]
